# revision 41
# baseline (speedup 1.0000x reference)
# Trainium2 Bass kernel for nn_FEDForecaster (FEDformer-style forecaster).
#
# Strategy: pure data-parallel over batch B=16 across 8 NeuronCores (2 batch
# elements per core); parameters replicated. The whole forward pass runs as a
# single Bass/Tile NEFF per core. All heavy matmuls run in bf16 (fp32 PSUM
# accumulation).
#
# Reformulations (validated numerically against the jax reference):
#   - series_decomp (pad+cumsum avgpool) == banded matrix A @ x
#   - rfft -> dense DFT matmuls Re = Ccos^T h, Im = Csin^T h (only top-16
#     needed downstream; amplitudes need the full spectrum anyway)
#   - top_k via 2 rounds of the DVE MAX8 + MATCH_REPLACE instructions,
#     producing a 0/1 frequency mask; gather/scatter are replaced by masked
#     attention over all 525 frequency rows (order-invariant).
#   - irfft of the 16-sparse conjugate-symmetric spectrum == B @ out with
#     B[t,f] = (c_f/N)(cos - sin), c_0 = c_{N/2} = 1 else 2.
#   - MHA softmax without max-subtraction (|scores| <= ~15, exp safe in f32);
#     row sums come free via a ones-column appended to V (out = [V;1]^T exp).
#   - freq-attention softmax keeps max-subtraction (|scores| ~ 1e5).
#   - trend path never materialized: pooled trend contribution reduces to
#     Win^T (mean_t trend), computed once in the input stage.
#   - 1/sqrt via ACT exp(-0.5 ln(v+eps)); 1/Z via DVE reciprocal_approx_fast.
#     Keeps the whole kernel on one ACT table set (exp/ln) - no table thrash.
#
# Activations live feature-on-partition ("transposed") so every matmul feeds
# the next without transposes, except h itself which is PE-transposed once per
# layer for the DFT (contraction over time).
#
# Scheduling: attention runs a lag-1 software pipeline (scores/exp of head
# pair hp+1 issue before the out-matmuls of hp; softmax normalization of
# chunk qc deferred into chunk qc+1) so the in-order PE queue never stalls
# on the ACT exp stream. Freq-block DFT constants are loaded once (not per
# layer). All DMAs use host-pre-tiled layouts (no strided descriptors).

import numpy as np
import ml_dtypes

B, T, F_IN = 16, 1024, 64
D, NHEAD, L, DFF = 512, 8, 2, 2048
OUT_LEN, N_TGT = 24, 8
KSIZE, TOPK = 25, 16
EPS = 1e-5
N = T + KSIZE - 1            # 1048
NF = N // 2 + 1              # 525
NCORES = 8
BPC = B // NCORES            # 2

# chunk tables
T9 = [(i * 128, min(128, N - i * 128)) for i in range(9)]       # t' partition chunks
NT3 = [(0, 512), (512, 512), (1024, 24)]                        # t free chunks
F5 = [(i * 128, min(128, NF - i * 128)) for i in range(5)]      # f partition chunks
NF2 = [(0, 512), (512, 13)]                                     # f free chunks

BF = ml_dtypes.bfloat16


def _host_constants():
    t_idx = np.arange(N)
    f_idx = np.arange(NF)
    theta = 2.0 * np.pi * np.outer(t_idx, f_idx) / N
    ccos = np.cos(theta)
    csin = np.sin(theta)
    cf = np.full(NF, 2.0)
    cf[0] = 1.0
    cf[NF - 1] = 1.0
    bmt = ((cf[None, :] / N) * (ccos - csin)).T                 # (525, 1048)
    a = np.zeros((N, T), np.float64)
    for tp in range(N):
        lo, hi = max(0, tp - (KSIZE - 1)), min(T - 1, tp)
        a[tp, lo:hi + 1] = 1.0 / KSIZE
    pos = np.arange(N, dtype=np.float32)[:, None]
    div = np.exp(np.arange(0, D, 2, dtype=np.float32) * (-np.log(10000.0) / D))
    pe = np.zeros((N, D), np.float32)
    pe[:, 0::2] = np.sin(pos * div)
    pe[:, 1::2] = np.cos(pos * div)

    def tile_p(m, nchunk):
        # (rows, cols) -> (128, nchunk, cols), zero-padding rows to 128*nchunk
        rows, cols = m.shape
        out = np.zeros((nchunk * 128, cols), m.dtype)
        out[:rows] = m
        return np.ascontiguousarray(
            out.reshape(nchunk, 128, cols).transpose(1, 0, 2))

    ccos_t = tile_p(ccos.astype(BF), 9)                          # (128, 9, 525)
    csin_t = tile_p(csin.astype(BF), 9)
    bmt_t = tile_p(bmt.astype(BF), 5)                            # (128, 5, 1048)
    at_t = tile_p(a.T.astype(BF), 8)                             # (128, 8, 1048)
    return ccos_t, csin_t, bmt_t, at_t, pe.T.copy()              # peT (512,1048) f32


def _build():
    import concourse.bass as bass
    import concourse.bacc as bacc
    import concourse.mybir as mybir
    import concourse.tile as tile
    from concourse import masks

    f32 = mybir.dt.float32
    bf16 = mybir.dt.bfloat16
    AF = mybir.ActivationFunctionType
    ALU = mybir.AluOpType
    AX = mybir.AxisListType

    nc = bacc.Bacc("TRN2", target_bir_lowering=False, debug=False)

    # ---------- DRAM I/O (all host-pre-tiled layouts) ----------
    d_x = nc.dram_tensor("x2", (BPC, 128, 8, F_IN), bf16, kind="ExternalInput").ap()
    d_win = nc.dram_tensor("win", (F_IN, D), bf16, kind="ExternalInput").ap()
    d_wqk = nc.dram_tensor("wqk", (L, 128, 4, 2 * D), bf16, kind="ExternalInput").ap()
    d_bqk = nc.dram_tensor("bqk", (L, 128, 8), f32, kind="ExternalInput").ap()
    d_wv = nc.dram_tensor("wv", (L, 128, 4, D), bf16, kind="ExternalInput").ap()
    d_bv = nc.dram_tensor("bv", (L, 1, D), bf16, kind="ExternalInput").ap()
    d_wo = nc.dram_tensor("wo", (L, 128, 4, D), bf16, kind="ExternalInput").ap()
    d_bo = nc.dram_tensor("bo", (L, 128, 4), f32, kind="ExternalInput").ap()
    d_w1 = nc.dram_tensor("w1", (L, 128, 4, DFF), bf16, kind="ExternalInput").ap()
    d_b1 = nc.dram_tensor("b1", (L, 128, 16), f32, kind="ExternalInput").ap()
    d_w2 = nc.dram_tensor("w2", (L, 128, 16, D), bf16, kind="ExternalInput").ap()
    d_b2 = nc.dram_tensor("b2", (L, 128, 4), f32, kind="ExternalInput").ap()
    d_g1 = nc.dram_tensor("g1", (L, 128, 4), f32, kind="ExternalInput").ap()
    d_be1 = nc.dram_tensor("be1", (L, 128, 4), f32, kind="ExternalInput").ap()
    d_g2 = nc.dram_tensor("g2", (L, 128, 4), f32, kind="ExternalInput").ap()
    d_be2 = nc.dram_tensor("be2", (L, 128, 4), f32, kind="ExternalInput").ap()
    d_fqw = nc.dram_tensor("fqw", (L, 128, 4, D), bf16, kind="ExternalInput").ap()
    d_fqb = nc.dram_tensor("fqb", (L, 128, 4), f32, kind="ExternalInput").ap()
    d_fkw = nc.dram_tensor("fkw", (L, 128, 4, D), bf16, kind="ExternalInput").ap()
    d_fkb = nc.dram_tensor("fkb", (L, 128, 4), f32, kind="ExternalInput").ap()
    d_fvw = nc.dram_tensor("fvw", (L, 128, 4, D), bf16, kind="ExternalInput").ap()
    d_fvb = nc.dram_tensor("fvb", (L, 1, D), bf16, kind="ExternalInput").ap()
    d_ccos = nc.dram_tensor("ccos", (128, 9, NF), bf16, kind="ExternalInput").ap()
    d_csin = nc.dram_tensor("csin", (128, 9, NF), bf16, kind="ExternalInput").ap()
    d_bmt = nc.dram_tensor("bmt", (128, 5, N), bf16, kind="ExternalInput").ap()
    d_at = nc.dram_tensor("at", (128, 8, N), bf16, kind="ExternalInput").ap()
    d_peb = nc.dram_tensor("peb", (128, 4, N), bf16, kind="ExternalInput").ap()
    d_hw = nc.dram_tensor("hw", (128, 4, OUT_LEN * N_TGT), f32, kind="ExternalInput").ap()
    d_hb = nc.dram_tensor("hb", (1, OUT_LEN * N_TGT), f32, kind="ExternalInput").ap()
    d_y = nc.dram_tensor("y", (BPC, OUT_LEN * N_TGT), f32, kind="ExternalOutput").ap()

    with tile.TileContext(nc) as tc:
        with tc.tile_pool(name="const", bufs=1) as constp, \
             tc.tile_pool(name="state", bufs=2) as statep, \
             tc.tile_pool(name="resid", bufs=2) as resp:

            ident = constp.tile([128, 128], bf16)
            masks.make_identity(nc, ident[:])
            ones_col = constp.tile([128, 1], bf16)
            nc.vector.memset(ones_col[:], 1.0)
            ones_row = constp.tile([1, 128], bf16)
            nc.vector.memset(ones_row[:], 1.0)
            ones2d = constp.tile([128, 64], bf16)
            nc.vector.memset(ones2d[:], 1.0)
            ones11f = constp.tile([1, 8], f32)
            nc.vector.memset(ones11f[:], 1.0)
            eps_r = constp.tile([1, 1], f32)
            nc.vector.memset(eps_r[:], EPS)
            eps_c = constp.tile([128, 1], f32)
            nc.vector.memset(eps_c[:], EPS)
            # per-batch pooled trend contribution, kept for the head stage
            pool2 = constp.tile([128, 2, 4], f32)

            h_st = {}    # live h tile per batch elem (feature-on-partition, bf16)

            def mm_acc(ps, pairs):
                for i, (lh, rh) in enumerate(pairs):
                    nc.tensor.matmul(ps, lh, rh, start=(i == 0),
                                     stop=(i == len(pairs) - 1))

            # ================= input stage =================
            with tc.tile_pool(name="inp", bufs=1) as ip, \
                 tc.tile_pool(name="inp2", bufs=2) as ip2, \
                 tc.tile_pool(name="ipsum", bufs=3, space=bass.MemorySpace.PSUM) as ips, \
                 tc.tile_pool(name="itr", bufs=2, space=bass.MemorySpace.PSUM) as itr:

                x_sbs = []
                for b in range(BPC):
                    x_sb = ip2.tile([128, 8, F_IN], bf16, tag="x", name="x_sb")
                    nc.sync.dma_start(out=x_sb[:], in_=d_x[b])
                    x_sbs.append(x_sb)
                at_sb = ip.tile([128, 8, N], bf16)
                nc.sync.dma_start(out=at_sb[:], in_=d_at)
                win_sb = ip.tile([64, D], bf16)
                nc.sync.dma_start(out=win_sb[:], in_=d_win)
                peb_sb = ip.tile([128, 4, N], bf16)
                nc.sync.dma_start(out=peb_sb[:], in_=d_peb)

                for b in range(BPC):
                    x_sb = x_sbs[b]
                    # xT (64, 1024) via 8 PE transposes
                    xt_sb = ip2.tile([64, T], bf16, tag="xt", name="xt_sb")
                    for tc8 in range(8):
                        pst = itr.tile([64, 128], bf16, tag="tp", name="pst")
                        nc.tensor.transpose(pst[:], x_sb[:, tc8, :], ident[:])
                        nc.scalar.copy(xt_sb[:, tc8 * 128:(tc8 + 1) * 128], pst[:])
                    # trendT (64, 1048) = lhsT=x_chunks, rhs=A^T
                    tr_sb = ip2.tile([64, N], bf16, tag="trend", name="tr_sb")
                    for (n0, nn) in NT3:
                        ps = ips.tile([64, 512], f32, tag="mm", name="ps")
                        mm_acc(ps[:, 0:nn],
                               [(x_sb[:, c, :], at_sb[:, c, n0:n0 + nn]) for c in range(8)])
                        nc.scalar.copy(tr_sb[:, n0:n0 + nn], ps[:, 0:nn])
                    # pooled trend term: Win^T (sum_t trend)  -> pool2[:, b, :]
                    trm_f = ip2.tile([64, 1], f32, tag="trmf", name="trm_f")
                    nc.vector.reduce_sum(trm_f[:], tr_sb[:], axis=AX.X)
                    trm = ip2.tile([64, 1], bf16, tag="trm", name="trm")
                    nc.vector.tensor_copy(trm[:], trm_f[:])
                    for m in range(4):
                        ps = ips.tile([128, 512], f32, tag="mm", name="ps")
                        nc.tensor.matmul(ps[:, 0:1], win_sb[:, m * 128:(m + 1) * 128],
                                         trm[:], start=True, stop=True)
                        nc.vector.tensor_copy(pool2[:, b, m:m + 1], ps[:, 0:1])
                    # s = pad(x) - trend  (64, N)
                    s_sb = ip2.tile([64, N], bf16, tag="s", name="s_sb")
                    nc.vector.tensor_scalar(s_sb[:, 0:12], tr_sb[:, 0:12],
                                            -1.0, None, op0=ALU.mult)
                    nc.vector.tensor_scalar(s_sb[:, 12 + T:N], tr_sb[:, 12 + T:N],
                                            -1.0, None, op0=ALU.mult)
                    nc.vector.tensor_sub(s_sb[:, 12:12 + T], xt_sb[:], tr_sb[:, 12:12 + T])
                    # h0 = Win^T s + peb
                    h0 = statep.tile([128, 4, N], bf16, tag=f"h{b}", name="h0")
                    for m in range(4):
                        for (n0, nn) in NT3:
                            ps = ips.tile([128, 512], f32, tag="mm", name="ps")
                            nc.tensor.matmul(ps[:, 0:nn], win_sb[:, m * 128:(m + 1) * 128],
                                             s_sb[:, n0:n0 + nn], start=True, stop=True)
                            nc.vector.tensor_add(h0[:, m, n0:n0 + nn], ps[:, 0:nn],
                                                 peb_sb[:, m, n0:n0 + nn])
                    h_st[b] = h0

            def _layernorm(sqp, mrow, pmm, potp, res, g_c, be_c, b):
                """LN over the feature (partition) axis of res (128,4,1048)."""
                sq = sqp.tile([128, 4, N], bf16, tag="sq", name="sq", bufs=1)
                nc.vector.tensor_mul(sq[:], res[:], res[:])
                mu_f = mrow.tile([1, N], f32, tag="muf", name="mu_f", bufs=1)
                va_f = mrow.tile([1, N], f32, tag="vaf", name="va_f", bufs=1)
                for (n0, nn) in NT3:
                    ps = pmm.tile([128, 512], f32, tag="mm", name="ps")
                    mm_acc(ps[0:1, 0:nn],
                           [(ones_col[:], res[:, j, n0:n0 + nn]) for j in range(4)])
                    nc.scalar.mul(mu_f[0:1, n0:n0 + nn], ps[0:1, 0:nn], 1.0 / D)
                    ps = pmm.tile([128, 512], f32, tag="mm", name="ps")
                    mm_acc(ps[0:1, 0:nn],
                           [(ones_col[:], sq[:, j, n0:n0 + nn]) for j in range(4)])
                    nc.scalar.mul(va_f[0:1, n0:n0 + nn], ps[0:1, 0:nn], 1.0 / D)
                mu_b = mrow.tile([1, N], bf16, tag="mub", name="mu_b")
                nc.vector.tensor_copy(mu_b[:], mu_f[:])
                tmp = mrow.tile([1, N], f32, tag="tmpf", name="tmp", bufs=1)
                nc.vector.tensor_mul(tmp[:], mu_f[:], mu_f[:])
                nc.vector.tensor_sub(tmp[:], va_f[:], tmp[:])
                nc.scalar.activation(va_f[:], tmp[:], AF.Ln, bias=eps_r[0:1, 0:1])
                rs_b = mrow.tile([1, N], bf16, tag="rsb", name="rs_b")
                nc.scalar.activation(rs_b[:], va_f[:], AF.Exp, scale=-0.5)
                hn = statep.tile([128, 4, N], bf16, tag=f"h{b}", name="hn")
                for (n0, nn) in NT3:
                    mub = potp.tile([128, 512], f32, tag="ot", name="mub")
                    nc.tensor.matmul(mub[:, 0:nn], ones_row[:],
                                     mu_b[0:1, n0:n0 + nn], start=True, stop=True)
                    rsb = potp.tile([128, 512], f32, tag="ot", name="rsb")
                    nc.tensor.matmul(rsb[:, 0:nn], ones_row[:],
                                     rs_b[0:1, n0:n0 + nn], start=True, stop=True)
                    for m in range(4):
                        nc.vector.tensor_sub(hn[:, m, n0:n0 + nn],
                                             res[:, m, n0:n0 + nn], mub[:, 0:nn])
                        nc.vector.tensor_mul(hn[:, m, n0:n0 + nn],
                                             hn[:, m, n0:n0 + nn], rsb[:, 0:nn])
                        nc.vector.tensor_scalar(hn[:, m, n0:n0 + nn],
                                                hn[:, m, n0:n0 + nn],
                                                g_c[:, m:m + 1], be_c[:, m:m + 1],
                                                op0=ALU.mult, op1=ALU.add)
                return hn

            # ================= layers =================
            for l in range(L):
                # ---------- MHA + LN1 ----------
                with tc.tile_pool(name="wmha", bufs=1) as wp, \
                     tc.tile_pool(name="amha", bufs=1) as ap_, \
                     tc.tile_pool(name="expp", bufs=2) as expp, \
                     tc.tile_pool(name="osbp", bufs=12) as osbp, \
                     tc.tile_pool(name="sqp", bufs=1) as sqp, \
                     tc.tile_pool(name="mrow", bufs=2) as mrow, \
                     tc.tile_pool(name="pmm", bufs=2, space=bass.MemorySpace.PSUM) as pmm, \
                     tc.tile_pool(name="psT", bufs=2, space=bass.MemorySpace.PSUM) as psT, \
                     tc.tile_pool(name="pot", bufs=2, space=bass.MemorySpace.PSUM) as pot:

                    wqk_sb = wp.tile([128, 4, 2 * D], bf16)
                    nc.sync.dma_start(out=wqk_sb[:], in_=d_wqk[l])
                    wv_sb = wp.tile([128, 4, D], bf16)
                    nc.sync.dma_start(out=wv_sb[:], in_=d_wv[l])
                    wo_sb = wp.tile([128, 4, D], bf16)
                    nc.sync.dma_start(out=wo_sb[:], in_=d_wo[l])
                    bqk_c = wp.tile([128, 8], f32)
                    nc.sync.dma_start(out=bqk_c[:], in_=d_bqk[l])
                    bv_r = wp.tile([1, D], bf16)
                    nc.sync.dma_start(out=bv_r[:], in_=d_bv[l])
                    bo_c = wp.tile([128, 4], f32)
                    nc.sync.dma_start(out=bo_c[:], in_=d_bo[l])
                    g1_c = wp.tile([128, 4], f32)
                    nc.sync.dma_start(out=g1_c[:], in_=d_g1[l])
                    be1_c = wp.tile([128, 4], f32)
                    nc.sync.dma_start(out=be1_c[:], in_=d_be1[l])

                    for b in range(BPC):
                        h = h_st[b]
                        # qkT (1024 feat, 1048 t)
                        qkT = ap_.tile([128, 8, N], bf16, tag="qkT", name="qkT")
                        for m8 in range(8):
                            for (n0, nn) in NT3:
                                ps = pmm.tile([128, 512], f32, tag="mm", name="ps")
                                mm_acc(ps[:, 0:nn],
                                       [(wqk_sb[:, j, m8 * 128:(m8 + 1) * 128],
                                         h[:, j, n0:n0 + nn]) for j in range(4)])
                                nc.scalar.activation(qkT[:, m8, n0:n0 + nn], ps[:, 0:nn],
                                                     AF.Identity, bias=bqk_c[:, m8:m8 + 1])
                        # v in natural layout (t', 8*65) with ones column per head
                        v_aug = ap_.tile([128, 9, 8 * 65], bf16, tag="vaug", name="v_aug")
                        for tc9, (t0, tn) in enumerate(T9):
                            ps = pmm.tile([128, 512], f32, tag="mm", name="ps")
                            for j in range(4):
                                nc.tensor.matmul(ps[0:tn, :], h[:, j, t0:t0 + tn],
                                                 wv_sb[:, j, :], start=(j == 0), stop=False)
                            nc.tensor.matmul(ps[0:tn, :], ones_row[0:1, 0:tn], bv_r[:],
                                             start=False, stop=True)
                            va = v_aug[0:tn, tc9, :].rearrange("p (h e) -> p h e", e=65)
                            nc.vector.tensor_copy(
                                va[:, :, 0:64],
                                ps[0:tn, :].rearrange("p (h e) -> p h e", e=64))
                            nc.vector.memset(va[:, :, 64:65], 1.0)

                        # ---- attention: lag-1 software pipeline ----
                        oT = ap_.tile([128, 4, N], bf16, tag="oT", name="oT")
                        st = {}   # per-qc: osb tiles, z tiles, rinv tiles

                        def scores_exp(qc, hp):
                            q0, qn = NT3[qc]
                            exP = expp.tile([128, 2, 9, 512], bf16, tag="exp", name="exP")
                            for tc9, (t0, tn) in enumerate(T9):
                                ps2 = psT.tile([128, 2, 512], f32, tag="st2", name="ps2")
                                for k in (0, 1):
                                    poff = 64 * k
                                    nc.tensor.matmul(ps2[0:tn, k, 0:qn],
                                                     qkT[poff:poff + 64, 4 + hp, t0:t0 + tn],
                                                     qkT[poff:poff + 64, hp, q0:q0 + qn],
                                                     start=True, stop=True)
                                nc.scalar.activation(exP[0:tn, :, tc9, 0:qn],
                                                     ps2[0:tn, :, 0:qn], AF.Exp)
                            return exP

                        def out_heads(qc, hp, exP):
                            q0, qn = NT3[qc]
                            if hp == 0:
                                st[qc] = {"osb": [None] * 8,
                                          "zr": [mrow.tile([97, 512], f32, tag="zra",
                                                           name="zra"),
                                                 mrow.tile([97, 512], f32, tag="zrb",
                                                           name="zrb")]}
                            for k in (0, 1):
                                hh = 2 * hp + k
                                po = pot.tile([65, 512], f32, tag="ot", name="po")
                                for i, (t0, tn) in enumerate(T9):
                                    nc.tensor.matmul(po[:, 0:qn],
                                                     v_aug[0:tn, i, 65 * hh:65 * hh + 65],
                                                     exP[0:tn, k, i, 0:qn],
                                                     start=(i == 0), stop=(i == 8))
                                osb = osbp.tile([65, 512], bf16, tag="osb", name="osb")
                                nc.vector.tensor_copy(osb[:, 0:qn], po[0:65, 0:qn])
                                zr = st[qc]["zr"][hh // 4]
                                r = 32 * (hh % 4)
                                nc.vector.tensor_copy(zr[r:r + 1, 0:qn], po[64:65, 0:qn])
                                st[qc]["osb"][hh] = osb

                        def znorm(qc):
                            q0, qn = NT3[qc]
                            rbs = []
                            for t in (0, 1):
                                rinv = mrow.tile([97, 512], f32, tag=f"rinv{t}",
                                                 name="rinv", bufs=1)
                                nc.vector.reciprocal_approx_fast(
                                    rinv[0:97, 0:qn], st[qc]["zr"][t][0:97, 0:qn])
                                rb = mrow.tile([97, 512], bf16, tag=f"rb{t}",
                                               name="rb", bufs=1)
                                nc.vector.tensor_copy(rb[0:97, 0:qn], rinv[0:97, 0:qn])
                                rbs.append(rb)
                            for hh in range(8):
                                r = 32 * (hh % 4)
                                pb = pmm.tile([128, 512], f32, tag="mm", name="pb")
                                nc.tensor.matmul(pb[0:64, 0:qn], ones2d[r:r + 1, 0:64],
                                                 rbs[hh // 4][r:r + 1, 0:qn],
                                                 start=True, stop=True,
                                                 tile_position=(r, 0))
                                poff = 64 * (hh % 2)
                                nc.vector.tensor_mul(oT[poff:poff + 64, hh // 2, q0:q0 + qn],
                                                     st[qc]["osb"][hh][0:64, 0:qn],
                                                     pb[0:64, 0:qn])
                            del st[qc]

                        steps = [(qc, hp) for qc in range(3) for hp in range(4)]
                        prev = None
                        zn_q = []
                        for (qc, hp) in steps:
                            exP = scores_exp(qc, hp)
                            if prev is not None:
                                out_heads(*prev)
                                if prev[1] == 3:
                                    zn_q.append(prev[0])
                            if zn_q and hp == 1 and qc != zn_q[0]:
                                znorm(zn_q.pop(0))
                            prev = (qc, hp, exP)
                        out_heads(*prev)
                        zn_q.append(prev[0])
                        for qc in zn_q:
                            znorm(qc)

                        # out-proj + residual, then LN1
                        res = resp.tile([128, 4, N], bf16, tag="res", name="res")
                        for m in range(4):
                            for (n0, nn) in NT3:
                                ps = pmm.tile([128, 512], f32, tag="mm", name="ps")
                                mm_acc(ps[:, 0:nn],
                                       [(wo_sb[:, j, m * 128:(m + 1) * 128],
                                         oT[:, j, n0:n0 + nn]) for j in range(4)])
                                nc.vector.tensor_add(res[:, m, n0:n0 + nn], ps[:, 0:nn],
                                                     h[:, m, n0:n0 + nn])
                                nc.vector.tensor_scalar(res[:, m, n0:n0 + nn],
                                                        res[:, m, n0:n0 + nn],
                                                        bo_c[:, m:m + 1], None,
                                                        op0=ALU.add)
                        h_st[b] = _layernorm(sqp, mrow, pmm, pot, res, g1_c, be1_c, b)

                # ---------- FF + LN2 (freq DFT consts prefetch during FF) ----------
                fcp = tc.alloc_tile_pool(name="fcst", bufs=1)
                ccos_sb = fcp.tile([128, 9, NF], bf16)
                nc.sync.dma_start(out=ccos_sb[:], in_=d_ccos)
                csin_sb = fcp.tile([128, 9, NF], bf16)
                nc.sync.dma_start(out=csin_sb[:], in_=d_csin)
                bmt_sb = fcp.tile([128, 5, N], bf16)
                nc.sync.dma_start(out=bmt_sb[:], in_=d_bmt)

                with tc.tile_pool(name="wff", bufs=1) as wp, \
                     tc.tile_pool(name="zp", bufs=2) as zp, \
                     tc.tile_pool(name="sqp2", bufs=1) as sqp, \
                     tc.tile_pool(name="mrow2", bufs=2) as mrow, \
                     tc.tile_pool(name="pmm2", bufs=4, space=bass.MemorySpace.PSUM) as pmm, \
                     tc.tile_pool(name="pot2", bufs=2, space=bass.MemorySpace.PSUM) as pot:

                    w1_sb = wp.tile([128, 4, DFF], bf16)
                    nc.sync.dma_start(out=w1_sb[:], in_=d_w1[l])
                    w2_sb = wp.tile([128, 16, D], bf16)
                    nc.sync.dma_start(out=w2_sb[:], in_=d_w2[l])
                    b1_c = wp.tile([128, 16], f32)
                    nc.sync.dma_start(out=b1_c[:], in_=d_b1[l])
                    b2_c = wp.tile([128, 4], f32)
                    nc.sync.dma_start(out=b2_c[:], in_=d_b2[l])
                    g2_c = wp.tile([128, 4], f32)
                    nc.sync.dma_start(out=g2_c[:], in_=d_g2[l])
                    be2_c = wp.tile([128, 4], f32)
                    nc.sync.dma_start(out=be2_c[:], in_=d_be2[l])

                    for b in range(BPC):
                        h1 = h_st[b]
                        res = resp.tile([128, 4, N], bf16, tag="res", name="res")
                        for (n0, nn) in NT3:
                            z_sb = zp.tile([128, 16, 512], bf16, tag="z", name="z_sb")
                            for m16 in range(16):
                                ps = pmm.tile([128, 512], f32, tag="mm", name="ps")
                                mm_acc(ps[:, 0:nn],
                                       [(w1_sb[:, j, m16 * 128:(m16 + 1) * 128],
                                         h1[:, j, n0:n0 + nn]) for j in range(4)])
                                nc.scalar.activation(z_sb[:, m16, 0:nn], ps[:, 0:nn],
                                                     AF.Relu, bias=b1_c[:, m16:m16 + 1])
                            for m in range(4):
                                ps = pmm.tile([128, 512], f32, tag="mm", name="ps")
                                mm_acc(ps[:, 0:nn],
                                       [(w2_sb[:, k, m * 128:(m + 1) * 128],
                                         z_sb[:, k, 0:nn]) for k in range(16)])
                                nc.vector.tensor_add(res[:, m, n0:n0 + nn], ps[:, 0:nn],
                                                     h1[:, m, n0:n0 + nn])
                                nc.vector.tensor_scalar(res[:, m, n0:n0 + nn],
                                                        res[:, m, n0:n0 + nn],
                                                        b2_c[:, m:m + 1], None,
                                                        op0=ALU.add)
                        h_st[b] = _layernorm(sqp, mrow, pmm, pot, res, g2_c, be2_c, b)

                # ---------- frequency block ----------
                with tc.tile_pool(name="wfr", bufs=1) as wp, \
                     tc.tile_pool(name="afr", bufs=1) as ap_, \
                     tc.tile_pool(name="afr2", bufs=1) as ap2, \
                     tc.tile_pool(name="frow", bufs=2) as frow, \
                     tc.tile_pool(name="pmm3", bufs=3, space=bass.MemorySpace.PSUM) as pmm, \
                     tc.tile_pool(name="ptr3", bufs=2, space=bass.MemorySpace.PSUM) as ptr, \
                     tc.tile_pool(name="pbc3", bufs=2, space=bass.MemorySpace.PSUM) as pbc:

                    fqw_sb = wp.tile([128, 4, D], bf16)
                    nc.sync.dma_start(out=fqw_sb[:], in_=d_fqw[l])
                    fkw_sb = wp.tile([128, 4, D], bf16)
                    nc.sync.dma_start(out=fkw_sb[:], in_=d_fkw[l])
                    fvw_sb = wp.tile([128, 4, D], bf16)
                    nc.sync.dma_start(out=fvw_sb[:], in_=d_fvw[l])
                    fqb_c = wp.tile([128, 4], f32)
                    nc.sync.dma_start(out=fqb_c[:], in_=d_fqb[l])
                    fkb_c = wp.tile([128, 4], f32)
                    nc.sync.dma_start(out=fkb_c[:], in_=d_fkb[l])
                    fvb_r = wp.tile([1, D], bf16)
                    nc.sync.dma_start(out=fvb_r[:], in_=d_fvb[l])

                    for b in range(BPC):
                        h2 = h_st[b]
                        # h in time-on-partition layout via PE transposes
                        htp = ap2.tile([128, 9, D], bf16, tag="htp", name="htp")
                        for tc9, (t0, tn) in enumerate(T9):
                            for j in range(4):
                                pst = ptr.tile([128, 128], bf16, tag="tp", name="pst")
                                nc.tensor.transpose(pst[0:tn, :], h2[:, j, t0:t0 + tn],
                                                    ident[:])
                                nc.scalar.copy(htp[0:tn, tc9, j * 128:(j + 1) * 128],
                                               pst[0:tn, :])
                        # DFT
                        reT = ap2.tile([128, 4, NF], bf16, tag="reT", name="reT")
                        imT = ap2.tile([128, 4, NF], bf16, tag="imT", name="imT")
                        for m in range(4):
                            for (f0, fn) in NF2:
                                ps = pmm.tile([128, 512], f32, tag="mm", name="ps")
                                mm_acc(ps[:, 0:fn],
                                       [(htp[0:tn, i, m * 128:(m + 1) * 128],
                                         ccos_sb[0:tn, i, f0:f0 + fn])
                                        for i, (t0, tn) in enumerate(T9)])
                                nc.scalar.copy(reT[:, m, f0:f0 + fn], ps[:, 0:fn])
                                ps = pmm.tile([128, 512], f32, tag="mm", name="ps")
                                mm_acc(ps[:, 0:fn],
                                       [(htp[0:tn, i, m * 128:(m + 1) * 128],
                                         csin_sb[0:tn, i, f0:f0 + fn])
                                        for i, (t0, tn) in enumerate(T9)])
                                nc.scalar.copy(imT[:, m, f0:f0 + fn], ps[:, 0:fn])
                        # amplitudes -> top-16 mask (sqrt via exp(0.5 ln))
                        absT = ap2.tile([128, 4, NF], bf16, tag="absT", name="absT")
                        tmpT = ap2.tile([128, 4, NF], bf16, tag="tmpT", name="tmpT")
                        lnT = ap2.tile([128, 4, NF], f32, tag="lnT", name="lnT")
                        nc.vector.tensor_mul(absT[:], reT[:], reT[:])
                        nc.vector.tensor_mul(tmpT[:], imT[:], imT[:])
                        nc.vector.tensor_add(absT[:], absT[:], tmpT[:])
                        # sqrt(x) = exp(0.5 ln(x + eps)); the +eps only regularizes
                        # near-zero amps and preserves the top-k ordering exactly
                        nc.scalar.activation(lnT[:], absT[:], AF.Ln, bias=eps_c[:, 0:1])
                        nc.scalar.activation(absT[:], lnT[:], AF.Exp, scale=0.5)
                        amp_row = frow.tile([1, NF], f32, tag="amp", name="amp_row")
                        for (f0, fn) in NF2:
                            ps = pmm.tile([128, 512], f32, tag="mm", name="ps")
                            mm_acc(ps[0:1, 0:fn],
                                   [(ones_col[:], absT[:, j, f0:f0 + fn]) for j in range(4)])
                            nc.scalar.copy(amp_row[0:1, f0:f0 + fn], ps[0:1, 0:fn])
                        work = frow.tile([1, NF], f32, tag="work", name="work")
                        nc.vector.tensor_copy(work[:], amp_row[:])
                        mx8 = frow.tile([1, 8], f32, tag="mx8", name="mx8")
                        for _ in range(2):
                            nc.vector.max(mx8[:], work[:])
                            nc.vector.match_replace(work[:], in_to_replace=mx8[:],
                                                    in_values=work[:], imm_value=0.0)
                        m_row = frow.tile([1, NF], f32, tag="mrow", name="m_row")
                        nc.vector.tensor_sub(m_row[:], amp_row[:], work[:])
                        nc.vector.tensor_scalar(m_row[:], m_row[:], 0.0, None, op0=ALU.is_gt)
                        pen_row = frow.tile([1, NF], bf16, tag="pen", name="pen_row")
                        nc.vector.tensor_scalar(pen_row[:], m_row[:], 1e9, -1e9,
                                                op0=ALU.mult, op1=ALU.add)
                        mb_row = frow.tile([1, NF], bf16, tag="mbrow", name="mb_row")
                        nc.vector.tensor_copy(mb_row[:], m_row[:])
                        # broadcast penalty row; mask column
                        pb_sb = ap2.tile([128, NF], f32, tag="pbsb", name="pb_sb")
                        for (f0, fn) in NF2:
                            pbp = pbc.tile([128, 512], f32, tag="bc", name="pbp")
                            nc.tensor.matmul(pbp[:, 0:fn], ones_row[:],
                                             pen_row[0:1, f0:f0 + fn], start=True, stop=True)
                            nc.vector.tensor_copy(pb_sb[:, f0:f0 + fn], pbp[:, 0:fn])
                        mcol = frow.tile([128, 5], f32, tag="mcol", name="mcol")
                        for fc, (f0, fn) in enumerate(F5):
                            pbp = pbc.tile([128, 512], f32, tag="bc", name="pbp")
                            nc.tensor.matmul(pbp[0:fn, 0:1], mb_row[0:1, f0:f0 + fn],
                                             ones_row[0:1, 0:1], start=True, stop=True)
                            nc.vector.tensor_copy(mcol[0:fn, fc:fc + 1], pbp[0:fn, 0:1])
                        # Q,K (feature-major) and V (freq-major)
                        qT = ap2.tile([128, 4, NF], bf16, tag="qT", name="qT")
                        kTf = ap2.tile([128, 4, NF], bf16, tag="kTf", name="kTf")
                        for m in range(4):
                            for (f0, fn) in NF2:
                                ps = pmm.tile([128, 512], f32, tag="mm", name="ps")
                                mm_acc(ps[:, 0:fn],
                                       [(fqw_sb[:, j, m * 128:(m + 1) * 128],
                                         reT[:, j, f0:f0 + fn]) for j in range(4)])
                                nc.scalar.activation(qT[:, m, f0:f0 + fn], ps[:, 0:fn],
                                                     AF.Identity, bias=fqb_c[:, m:m + 1])
                                ps = pmm.tile([128, 512], f32, tag="mm", name="ps")
                                mm_acc(ps[:, 0:fn],
                                       [(fkw_sb[:, j, m * 128:(m + 1) * 128],
                                         reT[:, j, f0:f0 + fn]) for j in range(4)])
                                nc.scalar.activation(kTf[:, m, f0:f0 + fn], ps[:, 0:fn],
                                                     AF.Identity, bias=fkb_c[:, m:m + 1])
                        v_sb = ap2.tile([128, 5, D], bf16, tag="vfr", name="v_sb")
                        for fc, (f0, fn) in enumerate(F5):
                            ps = pmm.tile([128, 512], f32, tag="mm", name="ps")
                            for j in range(4):
                                nc.tensor.matmul(ps[0:fn, :], reT[:, j, f0:f0 + fn],
                                                 fvw_sb[:, j, :], start=(j == 0), stop=False)
                            nc.tensor.matmul(ps[0:fn, :], ones_row[0:1, 0:fn], fvb_r[:],
                                             start=False, stop=True)
                            nc.scalar.copy(v_sb[0:fn, fc, :], ps[0:fn, :])
                        # masked scores -> softmax (with max subtraction)
                        sc_sb = ap_.tile([128, 5, NF], f32, tag="sc", name="sc_sb")
                        ex_sb = ap2.tile([128, 5, NF], bf16, tag="exf", name="ex_sb")
                        zcol = frow.tile([128, 5], f32, tag="zcol", name="zcol")
                        ncol = frow.tile([128, 5], f32, tag="ncol", name="ncol")
                        for qc, (q0, qn) in enumerate(F5):
                            for (f0, fn) in NF2:
                                ps = pmm.tile([128, 512], f32, tag="mm", name="ps")
                                mm_acc(ps[0:qn, 0:fn],
                                       [(qT[:, j, q0:q0 + qn], kTf[:, j, f0:f0 + fn])
                                        for j in range(4)])
                                nc.vector.tensor_add(sc_sb[0:qn, qc, f0:f0 + fn],
                                                     ps[0:qn, 0:fn], pb_sb[0:qn, f0:f0 + fn])
                            nc.vector.reduce_max(ncol[0:qn, qc:qc + 1], sc_sb[0:qn, qc, :],
                                                 axis=AX.X, negate=True)
                            nc.scalar.activation(ex_sb[0:qn, qc, :], sc_sb[0:qn, qc, :],
                                                 AF.Exp, bias=ncol[0:qn, qc:qc + 1],
                                                 accum_out=zcol[0:qn, qc:qc + 1])
                        rinv = frow.tile([128, 5], f32, tag="rinvf", name="rinv")
                        nc.vector.reciprocal_approx_fast(rinv[:], zcol[:])
                        wcol = frow.tile([128, 5], f32, tag="wcol", name="wcol")
                        nc.vector.tensor_mul(wcol[:], rinv[:], mcol[:])
                        # transpose exp -> (k_f, q_f)
                        exT = ap2.tile([128, 5, NF], bf16, tag="exT", name="exT")
                        for qc, (q0, qn) in enumerate(F5):
                            for fc, (f0, fn) in enumerate(F5):
                                pst = ptr.tile([128, 128], bf16, tag="tp", name="pst")
                                nc.tensor.transpose(pst[0:fn, 0:qn],
                                                    ex_sb[0:qn, qc, f0:f0 + fn],
                                                    ident[0:qn, 0:qn])
                                nc.scalar.copy(exT[0:fn, fc, q0:q0 + qn], pst[0:fn, 0:qn])
                        # ctx = attn @ V, masked+normalized
                        ctxm = ap2.tile([128, 5, D], bf16, tag="ctxm", name="ctxm")
                        for qc, (q0, qn) in enumerate(F5):
                            ps = pmm.tile([128, 512], f32, tag="mm", name="ps")
                            mm_acc(ps[0:qn, :],
                                   [(exT[0:fn, fc, q0:q0 + qn], v_sb[0:fn, fc, :])
                                    for fc, (f0, fn) in enumerate(F5)])
                            nc.vector.tensor_scalar(ctxm[0:qn, qc, :], ps[0:qn, :],
                                                    wcol[0:qn, qc:qc + 1], None,
                                                    op0=ALU.mult)
                        # irfft: h_next = B @ ctxm  (feature-major out)
                        hn = statep.tile([128, 4, N], bf16, tag=f"h{b}", name="hn")
                        for m in range(4):
                            for (n0, nn) in NT3:
                                ps = pmm.tile([128, 512], f32, tag="mm", name="ps")
                                mm_acc(ps[:, 0:nn],
                                       [(ctxm[0:fn, fc, m * 128:(m + 1) * 128],
                                         bmt_sb[0:fn, fc, n0:n0 + nn])
                                        for fc, (f0, fn) in enumerate(F5)])
                                nc.scalar.copy(hn[:, m, n0:n0 + nn], ps[:, 0:nn])
                        h_st[b] = hn
                fcp.release()

            # ================= head =================
            with tc.tile_pool(name="hd", bufs=1) as hp, \
                 tc.tile_pool(name="hd2", bufs=2) as hp2, \
                 tc.tile_pool(name="phd", bufs=2, space=bass.MemorySpace.PSUM) as php:
                hw_sb = hp.tile([128, 4, OUT_LEN * N_TGT], f32)
                nc.sync.dma_start(out=hw_sb[:], in_=d_hw)
                hb_sb = hp.tile([1, OUT_LEN * N_TGT], f32)
                nc.sync.dma_start(out=hb_sb[:], in_=d_hb)
                for b in range(BPC):
                    pool_c = hp2.tile([128, 4], f32, tag="pool", name="pool_c")
                    for m in range(4):
                        nc.vector.reduce_sum(pool_c[:, m:m + 1], h_st[b][:, m, :],
                                             axis=AX.X)
                    nc.vector.tensor_add(pool_c[:], pool_c[:], pool2[:, b, :])
                    ps = php.tile([1, OUT_LEN * N_TGT], f32, tag="y", name="ps")
                    for j in range(4):
                        nc.tensor.matmul(ps[:], pool_c[:, j:j + 1], hw_sb[:, j, :],
                                         start=(j == 0), stop=False)
                    nc.tensor.matmul(ps[:], ones11f[0:1, 0:1], hb_sb[:],
                                     start=False, stop=True)
                    y_sb = hp2.tile([1, OUT_LEN * N_TGT], f32, tag="ysb", name="y_sb")
                    nc.scalar.copy(y_sb[:], ps[:])
                    nc.sync.dma_start(out=d_y[b:b + 1, :], in_=y_sb[:])

    nc.compile()
    return nc


_NC_CACHE = {}


def _get_nc():
    if "nc" not in _NC_CACHE:
        _NC_CACHE["nc"] = _build()
    return _NC_CACHE["nc"]


def _tile_w(w):
    # (L, rows, cols) -> (L, 128, rows//128, cols)
    Lc, rows, cols = w.shape
    return np.ascontiguousarray(
        w.reshape(Lc, rows // 128, 128, cols).transpose(0, 2, 1, 3))


def _col_b(v):
    # (L, m*128) -> (L, 128, m)
    Lc, n = v.shape
    return np.ascontiguousarray(v.reshape(Lc, n // 128, 128).transpose(0, 2, 1))


def _prepare_in_maps(inputs):
    x = np.asarray(inputs["x"], np.float32)
    sq8 = 1.0 / 8.0
    sqD = 1.0 / np.sqrt(np.float32(D))
    qkv_w = np.asarray(inputs["qkv_w"], np.float32).copy()
    qkv_b = np.asarray(inputs["qkv_b"], np.float32).copy()
    qkv_w[:, :, :D] *= sq8
    qkv_b[:, :D] *= sq8
    fq_w = np.asarray(inputs["fq_w"], np.float32) * sqD
    fq_b = np.asarray(inputs["fq_b"], np.float32) * sqD
    ccos, csin, bmt, at, peT = _HOST_CONSTS
    b_in = np.asarray(inputs["b_in"], np.float32)
    peb = (peT + b_in[:, None]).astype(BF)                       # (512, 1048) bf16
    peb = np.ascontiguousarray(
        peb.reshape(4, 128, N).transpose(1, 0, 2))               # (128, 4, 1048)
    head_w = np.asarray(inputs["head_w"], np.float32)
    hb = (b_in @ head_w + np.asarray(inputs["head_b"], np.float32))[None, :]
    hw = head_w / np.float32(N)                                  # (512, 192)
    hw = np.ascontiguousarray(hw.reshape(4, 128, -1).transpose(1, 0, 2))

    common = {
        "win": np.asarray(inputs["Win"], np.float32).astype(BF),
        "wqk": _tile_w(qkv_w[:, :, :2 * D].astype(BF)),
        "bqk": _col_b(np.ascontiguousarray(qkv_b[:, :2 * D])),
        "wv": _tile_w(np.ascontiguousarray(qkv_w[:, :, 2 * D:]).astype(BF)),
        "bv": np.ascontiguousarray(qkv_b[:, None, 2 * D:]).astype(BF),
        "wo": _tile_w(np.asarray(inputs["out_w"], np.float32).astype(BF)),
        "bo": _col_b(np.asarray(inputs["out_b"], np.float32)),
        "w1": _tile_w(np.asarray(inputs["ff_w1"], np.float32).astype(BF)),
        "b1": _col_b(np.asarray(inputs["ff_b1"], np.float32)),
        "w2": _tile_w(np.asarray(inputs["ff_w2"], np.float32).astype(BF)),
        "b2": _col_b(np.asarray(inputs["ff_b2"], np.float32)),
        "g1": _col_b(np.asarray(inputs["ln1_g"], np.float32)),
        "be1": _col_b(np.asarray(inputs["ln1_b"], np.float32)),
        "g2": _col_b(np.asarray(inputs["ln2_g"], np.float32)),
        "be2": _col_b(np.asarray(inputs["ln2_b"], np.float32)),
        "fqw": _tile_w(fq_w.astype(BF)),
        "fqb": _col_b(fq_b),
        "fkw": _tile_w(np.asarray(inputs["fk_w"], np.float32).astype(BF)),
        "fkb": _col_b(np.asarray(inputs["fk_b"], np.float32)),
        "fvw": _tile_w(np.asarray(inputs["fv_w"], np.float32).astype(BF)),
        "fvb": np.asarray(inputs["fv_b"], np.float32)[:, None, :].astype(BF),
        "ccos": ccos, "csin": csin, "bmt": bmt, "at": at,
        "peb": peb, "hw": hw, "hb": hb,
    }
    in_maps = []
    for c in range(NCORES):
        m = dict(common)
        xc = x[c * BPC:(c + 1) * BPC].astype(BF)                 # (BPC, 1024, 64)
        m["x2"] = np.ascontiguousarray(
            xc.reshape(BPC, 8, 128, F_IN).transpose(0, 2, 1, 3))
        in_maps.append(m)
    return in_maps


def kernel(**inputs):
    in_maps = _prepare_in_maps(inputs)
    from concourse.bass_utils import run_bass_kernel_spmd
    nc = _get_nc()
    res = run_bass_kernel_spmd(nc, in_maps, core_ids=list(range(NCORES)))
    ys = np.concatenate([res.results[c]["y"] for c in range(NCORES)], axis=0)
    return ys.reshape(B, OUT_LEN, N_TGT).astype(np.float32)


_HOST_CONSTS = _host_constants()


# revision 43
# speedup vs baseline: 1.0453x; 1.0453x over previous
# Trainium2 Bass kernel for nn_FEDForecaster (FEDformer-style forecaster).
#
# Strategy: pure data-parallel over batch B=16 across 8 NeuronCores (2 batch
# elements per core); parameters replicated. The whole forward pass runs as a
# single Bass/Tile NEFF per core. All heavy matmuls run in bf16 (fp32 PSUM
# accumulation).
#
# Reformulations (validated numerically against the jax reference):
#   - series_decomp (pad+cumsum avgpool) == banded matrix A @ x
#   - rfft -> dense DFT matmuls Re = Ccos^T h, Im = Csin^T h (only top-16
#     needed downstream; amplitudes need the full spectrum anyway)
#   - top_k via 2 rounds of the DVE MAX8 + MATCH_REPLACE instructions,
#     producing a 0/1 frequency mask; gather/scatter are replaced by masked
#     attention over all 525 frequency rows (order-invariant).
#   - irfft of the 16-sparse conjugate-symmetric spectrum == B @ out with
#     B[t,f] = (c_f/N)(cos - sin), c_0 = c_{N/2} = 1 else 2.
#   - MHA softmax without max-subtraction (|scores| <= ~15, exp safe in f32);
#     row sums come free via a ones-column appended to V (out = [V;1]^T exp).
#   - freq-attention softmax keeps max-subtraction (|scores| ~ 1e5).
#   - trend path never materialized: pooled trend contribution reduces to
#     Win^T (mean_t trend), computed once in the input stage.
#   - 1/sqrt via ACT exp(-0.5 ln(v+eps)); 1/Z via DVE reciprocal_approx_fast.
#     Keeps the whole kernel on one ACT table set (exp/ln) - no table thrash.
#
# Activations live feature-on-partition ("transposed") so every matmul feeds
# the next without transposes, except h itself which is PE-transposed once per
# layer for the DFT (contraction over time).
#
# Scheduling: attention runs a lag-1 software pipeline (scores/exp of head
# pair hp+1 issue before the out-matmuls of hp; softmax normalization of
# chunk qc deferred into chunk qc+1) so the in-order PE queue never stalls
# on the ACT exp stream. Freq-block DFT constants are loaded once (not per
# layer). All DMAs use host-pre-tiled layouts (no strided descriptors).

import numpy as np
import ml_dtypes

B, T, F_IN = 16, 1024, 64
D, NHEAD, L, DFF = 512, 8, 2, 2048
OUT_LEN, N_TGT = 24, 8
KSIZE, TOPK = 25, 16
EPS = 1e-5
N = T + KSIZE - 1            # 1048
NF = N // 2 + 1              # 525
NCORES = 8
BPC = B // NCORES            # 2

# chunk tables
T9 = [(i * 128, min(128, N - i * 128)) for i in range(9)]       # t' partition chunks
NT3 = [(0, 512), (512, 512), (1024, 24)]                        # t free chunks
F5 = [(i * 128, min(128, NF - i * 128)) for i in range(5)]      # f partition chunks
NF2 = [(0, 512), (512, 13)]                                     # f free chunks

BF = ml_dtypes.bfloat16


def _host_constants():
    t_idx = np.arange(N)
    f_idx = np.arange(NF)
    theta = 2.0 * np.pi * np.outer(t_idx, f_idx) / N
    ccos = np.cos(theta)
    csin = np.sin(theta)
    cf = np.full(NF, 2.0)
    cf[0] = 1.0
    cf[NF - 1] = 1.0
    bmt = ((cf[None, :] / N) * (ccos - csin)).T                 # (525, 1048)
    a = np.zeros((N, T), np.float64)
    for tp in range(N):
        lo, hi = max(0, tp - (KSIZE - 1)), min(T - 1, tp)
        a[tp, lo:hi + 1] = 1.0 / KSIZE
    pos = np.arange(N, dtype=np.float32)[:, None]
    div = np.exp(np.arange(0, D, 2, dtype=np.float32) * (-np.log(10000.0) / D))
    pe = np.zeros((N, D), np.float32)
    pe[:, 0::2] = np.sin(pos * div)
    pe[:, 1::2] = np.cos(pos * div)

    def tile_p(m, nchunk):
        # (rows, cols) -> (128, nchunk, cols), zero-padding rows to 128*nchunk
        rows, cols = m.shape
        out = np.zeros((nchunk * 128, cols), m.dtype)
        out[:rows] = m
        return np.ascontiguousarray(
            out.reshape(nchunk, 128, cols).transpose(1, 0, 2))

    ccos_t = tile_p(ccos.astype(BF), 9)                          # (128, 9, 525)
    csin_t = tile_p(csin.astype(BF), 9)
    bmt_t = tile_p(bmt.astype(BF), 5)                            # (128, 5, 1048)
    at_t = tile_p(a.T.astype(BF), 8)                             # (128, 8, 1048)
    bsum_t = tile_p(bmt.sum(axis=1).astype(BF)[:, None], 5)      # (128, 5, 1)
    return ccos_t, csin_t, bmt_t, at_t, bsum_t, pe.T.copy()      # peT (512,1048) f32


def _build():
    import concourse.bass as bass
    import concourse.bacc as bacc
    import concourse.mybir as mybir
    import concourse.tile as tile
    from concourse import masks

    f32 = mybir.dt.float32
    bf16 = mybir.dt.bfloat16
    AF = mybir.ActivationFunctionType
    ALU = mybir.AluOpType
    AX = mybir.AxisListType

    nc = bacc.Bacc("TRN2", target_bir_lowering=False, debug=False)

    # Steer the ACT table-load pass to the combined ln/exp set: without this,
    # Exp resolves to `exp_and_others` and Ln to `natural_log`, and every
    # LayerNorm pays two ~2.7us table switches. Restricting (not changing) the
    # advertised contents of the other sets makes `natural_log_exp_and_others`
    # the unique provider of both, so one table load covers the whole kernel.
    from concourse.hw_specs import get_activation_tables
    _tabs = get_activation_tables(nc.m.arch)
    if "natural_log_exp_and_others" in _tabs:
        for _nm, _s in _tabs.items():
            if _nm != "natural_log_exp_and_others":
                _s.discard(AF.Exp)
                _s.discard(AF.Ln)

    # ---------- DRAM I/O (all host-pre-tiled layouts) ----------
    d_x = nc.dram_tensor("x2", (BPC, 128, 8, F_IN), bf16, kind="ExternalInput").ap()
    d_win = nc.dram_tensor("win", (F_IN, D), bf16, kind="ExternalInput").ap()
    d_wqk = nc.dram_tensor("wqk", (L, 128, 4, 2 * D), bf16, kind="ExternalInput").ap()
    d_bqk = nc.dram_tensor("bqk", (L, 128, 8), f32, kind="ExternalInput").ap()
    d_wv = nc.dram_tensor("wv", (L, 128, 4, D), bf16, kind="ExternalInput").ap()
    d_bv = nc.dram_tensor("bv", (L, 1, D), bf16, kind="ExternalInput").ap()
    d_wo = nc.dram_tensor("wo", (L, 128, 4, D), bf16, kind="ExternalInput").ap()
    d_bo = nc.dram_tensor("bo", (L, 128, 4), f32, kind="ExternalInput").ap()
    d_w1 = nc.dram_tensor("w1", (L, 128, 4, DFF), bf16, kind="ExternalInput").ap()
    d_b1 = nc.dram_tensor("b1", (L, 128, 16), f32, kind="ExternalInput").ap()
    d_w2 = nc.dram_tensor("w2", (L, 128, 16, D), bf16, kind="ExternalInput").ap()
    d_b2 = nc.dram_tensor("b2", (L, 128, 4), f32, kind="ExternalInput").ap()
    d_g1 = nc.dram_tensor("g1", (L, 128, 4), f32, kind="ExternalInput").ap()
    d_be1 = nc.dram_tensor("be1", (L, 128, 4), f32, kind="ExternalInput").ap()
    d_g2 = nc.dram_tensor("g2", (L, 128, 4), f32, kind="ExternalInput").ap()
    d_be2 = nc.dram_tensor("be2", (L, 128, 4), f32, kind="ExternalInput").ap()
    d_fqw = nc.dram_tensor("fqw", (L, 128, 4, D), bf16, kind="ExternalInput").ap()
    d_fqb = nc.dram_tensor("fqb", (L, 128, 4), f32, kind="ExternalInput").ap()
    d_fkw = nc.dram_tensor("fkw", (L, 128, 4, D), bf16, kind="ExternalInput").ap()
    d_fkb = nc.dram_tensor("fkb", (L, 128, 4), f32, kind="ExternalInput").ap()
    d_fvw = nc.dram_tensor("fvw", (L, 128, 4, D), bf16, kind="ExternalInput").ap()
    d_fvb = nc.dram_tensor("fvb", (L, 1, D), bf16, kind="ExternalInput").ap()
    d_ccos = nc.dram_tensor("ccos", (128, 9, NF), bf16, kind="ExternalInput").ap()
    d_csin = nc.dram_tensor("csin", (128, 9, NF), bf16, kind="ExternalInput").ap()
    d_bmt = nc.dram_tensor("bmt", (128, 5, N), bf16, kind="ExternalInput").ap()
    d_bsum = nc.dram_tensor("bsum", (128, 5, 1), bf16, kind="ExternalInput").ap()
    d_at = nc.dram_tensor("at", (128, 8, N), bf16, kind="ExternalInput").ap()
    d_peb = nc.dram_tensor("peb", (128, 4, N), bf16, kind="ExternalInput").ap()
    d_hw = nc.dram_tensor("hw", (128, 4, OUT_LEN * N_TGT), f32, kind="ExternalInput").ap()
    d_hb = nc.dram_tensor("hb", (1, OUT_LEN * N_TGT), f32, kind="ExternalInput").ap()
    d_y = nc.dram_tensor("y", (BPC, OUT_LEN * N_TGT), f32, kind="ExternalOutput").ap()

    with tile.TileContext(nc) as tc:
        with tc.tile_pool(name="const", bufs=1) as constp, \
             tc.tile_pool(name="state", bufs=2) as statep, \
             tc.tile_pool(name="resid", bufs=2) as resp:

            ident = constp.tile([128, 128], bf16)
            masks.make_identity(nc, ident[:])
            ones_col = constp.tile([128, 1], bf16)
            nc.vector.memset(ones_col[:], 1.0)
            ones_row = constp.tile([1, 128], bf16)
            nc.vector.memset(ones_row[:], 1.0)
            ones2d = constp.tile([128, 64], bf16)
            nc.vector.memset(ones2d[:], 1.0)
            ones11f = constp.tile([1, 8], f32)
            nc.vector.memset(ones11f[:], 1.0)
            eps_r = constp.tile([1, 1], f32)
            nc.vector.memset(eps_r[:], EPS)
            eps_c = constp.tile([128, 1], f32)
            nc.vector.memset(eps_c[:], EPS)
            # per-batch pooled trend contribution, kept for the head stage
            pool2 = constp.tile([128, 2, 4], f32)
            # per-batch pooled seasonal output (written by the last freq block)
            poolh = constp.tile([128, 2, 4], f32)

            h_st = {}    # live h tile per batch elem (feature-on-partition, bf16)

            def mm_acc(ps, pairs):
                for i, (lh, rh) in enumerate(pairs):
                    nc.tensor.matmul(ps, lh, rh, start=(i == 0),
                                     stop=(i == len(pairs) - 1))

            # ================= input stage =================
            with tc.tile_pool(name="inp", bufs=1) as ip, \
                 tc.tile_pool(name="inp2", bufs=2) as ip2, \
                 tc.tile_pool(name="ipsum", bufs=3, space=bass.MemorySpace.PSUM) as ips, \
                 tc.tile_pool(name="itr", bufs=2, space=bass.MemorySpace.PSUM) as itr:

                x_sbs = []
                for b in range(BPC):
                    x_sb = ip2.tile([128, 8, F_IN], bf16, tag="x", name="x_sb")
                    nc.sync.dma_start(out=x_sb[:], in_=d_x[b])
                    x_sbs.append(x_sb)
                at_sb = ip.tile([128, 8, N], bf16)
                nc.sync.dma_start(out=at_sb[:], in_=d_at)
                win_sb = ip.tile([64, D], bf16)
                nc.sync.dma_start(out=win_sb[:], in_=d_win)
                peb_sb = ip.tile([128, 4, N], bf16)
                nc.sync.dma_start(out=peb_sb[:], in_=d_peb)

                for b in range(BPC):
                    x_sb = x_sbs[b]
                    # xT (64, 1024) via 8 PE transposes
                    xt_sb = ip2.tile([64, T], bf16, tag="xt", name="xt_sb")
                    for tc8 in range(8):
                        pst = itr.tile([64, 128], bf16, tag="tp", name="pst")
                        nc.tensor.transpose(pst[:], x_sb[:, tc8, :], ident[:])
                        nc.scalar.copy(xt_sb[:, tc8 * 128:(tc8 + 1) * 128], pst[:])
                    # trendT (64, 1048) = lhsT=x_chunks, rhs=A^T
                    tr_sb = ip2.tile([64, N], bf16, tag="trend", name="tr_sb")
                    for (n0, nn) in NT3:
                        ps = ips.tile([64, 512], f32, tag="mm", name="ps")
                        mm_acc(ps[:, 0:nn],
                               [(x_sb[:, c, :], at_sb[:, c, n0:n0 + nn]) for c in range(8)])
                        nc.scalar.copy(tr_sb[:, n0:n0 + nn], ps[:, 0:nn])
                    # pooled trend term: Win^T (sum_t trend)  -> pool2[:, b, :]
                    trm_f = ip2.tile([64, 1], f32, tag="trmf", name="trm_f")
                    nc.vector.reduce_sum(trm_f[:], tr_sb[:], axis=AX.X)
                    trm = ip2.tile([64, 1], bf16, tag="trm", name="trm")
                    nc.vector.tensor_copy(trm[:], trm_f[:])
                    for m in range(4):
                        ps = ips.tile([128, 512], f32, tag="mm", name="ps")
                        nc.tensor.matmul(ps[:, 0:1], win_sb[:, m * 128:(m + 1) * 128],
                                         trm[:], start=True, stop=True)
                        nc.vector.tensor_copy(pool2[:, b, m:m + 1], ps[:, 0:1])
                    # s = pad(x) - trend  (64, N)
                    s_sb = ip2.tile([64, N], bf16, tag="s", name="s_sb")
                    nc.vector.tensor_scalar(s_sb[:, 0:12], tr_sb[:, 0:12],
                                            -1.0, None, op0=ALU.mult)
                    nc.vector.tensor_scalar(s_sb[:, 12 + T:N], tr_sb[:, 12 + T:N],
                                            -1.0, None, op0=ALU.mult)
                    nc.vector.tensor_sub(s_sb[:, 12:12 + T], xt_sb[:], tr_sb[:, 12:12 + T])
                    # h0 = Win^T s + peb
                    h0 = statep.tile([128, 4, N], bf16, tag=f"h{b}", name="h0")
                    for m in range(4):
                        for (n0, nn) in NT3:
                            ps = ips.tile([128, 512], f32, tag="mm", name="ps")
                            nc.tensor.matmul(ps[:, 0:nn], win_sb[:, m * 128:(m + 1) * 128],
                                             s_sb[:, n0:n0 + nn], start=True, stop=True)
                            nc.vector.tensor_add(h0[:, m, n0:n0 + nn], ps[:, 0:nn],
                                                 peb_sb[:, m, n0:n0 + nn])
                    h_st[b] = h0

            def _layernorm(sqp, mrow, pmm, potp, res, g_c, be_c, b):
                """LN over the feature (partition) axis of res (128,4,1048)."""
                sq = sqp.tile([128, 4, N], bf16, tag="sq", name="sq", bufs=1)
                nc.vector.tensor_mul(sq[:], res[:], res[:])
                mu_f = mrow.tile([1, N], f32, tag="muf", name="mu_f", bufs=1)
                va_f = mrow.tile([1, N], f32, tag="vaf", name="va_f", bufs=1)
                for (n0, nn) in NT3:
                    ps = pmm.tile([128, 512], f32, tag="mm", name="ps")
                    mm_acc(ps[0:1, 0:nn],
                           [(ones_col[:], res[:, j, n0:n0 + nn]) for j in range(4)])
                    nc.scalar.mul(mu_f[0:1, n0:n0 + nn], ps[0:1, 0:nn], 1.0 / D)
                    ps = pmm.tile([128, 512], f32, tag="mm", name="ps")
                    mm_acc(ps[0:1, 0:nn],
                           [(ones_col[:], sq[:, j, n0:n0 + nn]) for j in range(4)])
                    nc.scalar.mul(va_f[0:1, n0:n0 + nn], ps[0:1, 0:nn], 1.0 / D)
                mu_b = mrow.tile([1, N], bf16, tag="mub", name="mu_b")
                nc.vector.tensor_copy(mu_b[:], mu_f[:])
                tmp = mrow.tile([1, N], f32, tag="tmpf", name="tmp", bufs=1)
                nc.vector.tensor_mul(tmp[:], mu_f[:], mu_f[:])
                nc.vector.tensor_sub(tmp[:], va_f[:], tmp[:])
                nc.scalar.activation(va_f[:], tmp[:], AF.Ln, bias=eps_r[0:1, 0:1])
                rs_b = mrow.tile([1, N], bf16, tag="rsb", name="rs_b")
                nc.scalar.activation(rs_b[:], va_f[:], AF.Exp, scale=-0.5)
                hn = statep.tile([128, 4, N], bf16, tag=f"h{b}", name="hn")
                for ci, (n0, nn) in enumerate(NT3):
                    _ln_apply_chunk(mrow, potp, hn, res, mu_b, rs_b, g_c, be_c, ci)
                return hn

            def _ln_apply_chunk(mrow, potp, hn, res, mu_b, rs_b, g_c, be_c, ci):
                # broadcast mu/rs rows, stage to SBUF bf16 (ACT) so the three
                # elementwise ops run in DVE 2x mode instead of 1x PSUM mode
                (n0, nn) = NT3[ci]
                mub = potp.tile([128, 512], f32, tag="ot", name="mub")
                nc.tensor.matmul(mub[:, 0:nn], ones_row[:],
                                 mu_b[0:1, n0:n0 + nn], start=True, stop=True)
                rsb = potp.tile([128, 512], f32, tag="ot", name="rsb")
                nc.tensor.matmul(rsb[:, 0:nn], ones_row[:],
                                 rs_b[0:1, n0:n0 + nn], start=True, stop=True)
                mus = mrow.tile([128, 512], bf16, tag="mus", name="mus")
                nc.scalar.copy(mus[:, 0:nn], mub[:, 0:nn])
                rss = mrow.tile([128, 512], bf16, tag="rss", name="rss")
                nc.scalar.copy(rss[:, 0:nn], rsb[:, 0:nn])
                for m in range(4):
                    nc.vector.tensor_sub(hn[:, m, n0:n0 + nn],
                                         res[:, m, n0:n0 + nn], mus[:, 0:nn])
                    nc.vector.tensor_mul(hn[:, m, n0:n0 + nn],
                                         hn[:, m, n0:n0 + nn], rss[:, 0:nn])
                    nc.vector.tensor_scalar(hn[:, m, n0:n0 + nn],
                                            hn[:, m, n0:n0 + nn],
                                            g_c[:, m:m + 1], be_c[:, m:m + 1],
                                            op0=ALU.mult, op1=ALU.add)

            # ================= layers =================
            for l in range(L):
                # ---------- MHA + LN1 ----------
                with tc.tile_pool(name="wmha", bufs=1) as wp, \
                     tc.tile_pool(name="amha", bufs=1) as ap_, \
                     tc.tile_pool(name="expp", bufs=2) as expp, \
                     tc.tile_pool(name="osbp", bufs=12) as osbp, \
                     tc.tile_pool(name="sqp", bufs=1) as sqp, \
                     tc.tile_pool(name="mrow", bufs=2) as mrow, \
                     tc.tile_pool(name="pmm", bufs=2, space=bass.MemorySpace.PSUM) as pmm, \
                     tc.tile_pool(name="psT", bufs=2, space=bass.MemorySpace.PSUM) as psT, \
                     tc.tile_pool(name="pot", bufs=2, space=bass.MemorySpace.PSUM) as pot:

                    wqk_sb = wp.tile([128, 4, 2 * D], bf16)
                    nc.sync.dma_start(out=wqk_sb[:], in_=d_wqk[l])
                    wv_sb = wp.tile([128, 4, D], bf16)
                    nc.sync.dma_start(out=wv_sb[:], in_=d_wv[l])
                    wo_sb = wp.tile([128, 4, D], bf16)
                    nc.sync.dma_start(out=wo_sb[:], in_=d_wo[l])
                    bqk_c = wp.tile([128, 8], f32)
                    nc.sync.dma_start(out=bqk_c[:], in_=d_bqk[l])
                    bv_r = wp.tile([1, D], bf16)
                    nc.sync.dma_start(out=bv_r[:], in_=d_bv[l])
                    bo_c = wp.tile([128, 4], f32)
                    nc.sync.dma_start(out=bo_c[:], in_=d_bo[l])
                    g1_c = wp.tile([128, 4], f32)
                    nc.sync.dma_start(out=g1_c[:], in_=d_g1[l])
                    be1_c = wp.tile([128, 4], f32)
                    nc.sync.dma_start(out=be1_c[:], in_=d_be1[l])

                    pend_ln = None   # LN of batch elem b hides under qkT/v of b+1
                    for b in range(BPC):
                        h = h_st[b]
                        # qkT (1024 feat, 1048 t)
                        qkT = ap_.tile([128, 8, N], bf16, tag="qkT", name="qkT")
                        for m8 in range(8):
                            for (n0, nn) in NT3:
                                ps = pmm.tile([128, 512], f32, tag="mm", name="ps")
                                mm_acc(ps[:, 0:nn],
                                       [(wqk_sb[:, j, m8 * 128:(m8 + 1) * 128],
                                         h[:, j, n0:n0 + nn]) for j in range(4)])
                                nc.scalar.activation(qkT[:, m8, n0:n0 + nn], ps[:, 0:nn],
                                                     AF.Identity, bias=bqk_c[:, m8:m8 + 1])
                        # v in natural layout (t', 8*65) with ones column per head
                        v_aug = ap_.tile([128, 9, 8 * 65], bf16, tag="vaug", name="v_aug")
                        for tc9, (t0, tn) in enumerate(T9):
                            ps = pmm.tile([128, 512], f32, tag="mm", name="ps")
                            for j in range(4):
                                nc.tensor.matmul(ps[0:tn, :], h[:, j, t0:t0 + tn],
                                                 wv_sb[:, j, :], start=(j == 0), stop=False)
                            nc.tensor.matmul(ps[0:tn, :], ones_row[0:1, 0:tn], bv_r[:],
                                             start=False, stop=True)
                            va = v_aug[0:tn, tc9, :].rearrange("p (h e) -> p h e", e=65)
                            nc.vector.tensor_copy(
                                va[:, :, 0:64],
                                ps[0:tn, :].rearrange("p (h e) -> p h e", e=64))
                            nc.vector.memset(va[:, :, 64:65], 1.0)

                        if pend_ln is not None:
                            h_st[pend_ln[1]] = _layernorm(
                                sqp, mrow, pmm, pot, pend_ln[0], g1_c, be1_c,
                                pend_ln[1])

                        # ---- attention: lag-1 software pipeline ----
                        oT = ap_.tile([128, 4, N], bf16, tag="oT", name="oT")
                        st = {}   # per-qc: osb tiles, z tiles, rinv tiles

                        def scores_exp(qc, hp):
                            q0, qn = NT3[qc]
                            exP = expp.tile([128, 2, 9, 512], bf16, tag="exp", name="exP")
                            for tc9, (t0, tn) in enumerate(T9):
                                ps2 = psT.tile([128, 2, 512], f32, tag="st2", name="ps2")
                                for k in (0, 1):
                                    poff = 64 * k
                                    nc.tensor.matmul(ps2[0:tn, k, 0:qn],
                                                     qkT[poff:poff + 64, 4 + hp, t0:t0 + tn],
                                                     qkT[poff:poff + 64, hp, q0:q0 + qn],
                                                     start=True, stop=True)
                                nc.scalar.activation(exP[0:tn, :, tc9, 0:qn],
                                                     ps2[0:tn, :, 0:qn], AF.Exp)
                            return exP

                        def out_heads(qc, hp, exP):
                            q0, qn = NT3[qc]
                            if hp == 0:
                                st[qc] = {"osb": [None] * 8,
                                          "zr": [mrow.tile([97, 512], f32, tag="zra",
                                                           name="zra"),
                                                 mrow.tile([97, 512], f32, tag="zrb",
                                                           name="zrb")]}
                            for k in (0, 1):
                                hh = 2 * hp + k
                                po = pot.tile([65, 512], f32, tag="ot", name="po")
                                for i, (t0, tn) in enumerate(T9):
                                    nc.tensor.matmul(po[:, 0:qn],
                                                     v_aug[0:tn, i, 65 * hh:65 * hh + 65],
                                                     exP[0:tn, k, i, 0:qn],
                                                     start=(i == 0), stop=(i == 8))
                                osb = osbp.tile([65, 512], bf16, tag="osb", name="osb")
                                nc.vector.tensor_copy(osb[:, 0:qn], po[0:65, 0:qn])
                                zr = st[qc]["zr"][hh // 4]
                                r = 32 * (hh % 4)
                                nc.vector.tensor_copy(zr[r:r + 1, 0:qn], po[64:65, 0:qn])
                                st[qc]["osb"][hh] = osb

                        def znorm(qc):
                            q0, qn = NT3[qc]
                            rbs = []
                            for t in (0, 1):
                                rinv = mrow.tile([97, 512], f32, tag=f"rinv{t}",
                                                 name="rinv", bufs=1)
                                nc.vector.reciprocal_approx_fast(
                                    rinv[0:97, 0:qn], st[qc]["zr"][t][0:97, 0:qn])
                                rb = mrow.tile([97, 512], bf16, tag=f"rb{t}",
                                               name="rb", bufs=1)
                                nc.vector.tensor_copy(rb[0:97, 0:qn], rinv[0:97, 0:qn])
                                rbs.append(rb)
                            for hh in range(8):
                                r = 32 * (hh % 4)
                                pb = pmm.tile([128, 512], f32, tag="mm", name="pb")
                                nc.tensor.matmul(pb[0:64, 0:qn], ones2d[r:r + 1, 0:64],
                                                 rbs[hh // 4][r:r + 1, 0:qn],
                                                 start=True, stop=True,
                                                 tile_position=(r, 0))
                                poff = 64 * (hh % 2)
                                nc.vector.tensor_mul(oT[poff:poff + 64, hh // 2, q0:q0 + qn],
                                                     st[qc]["osb"][hh][0:64, 0:qn],
                                                     pb[0:64, 0:qn])
                            del st[qc]

                        steps = [(qc, hp) for qc in range(3) for hp in range(4)]
                        prev = None
                        zn_q = []
                        for (qc, hp) in steps:
                            exP = scores_exp(qc, hp)
                            if prev is not None:
                                out_heads(*prev)
                                if prev[1] == 3:
                                    zn_q.append(prev[0])
                            if zn_q and hp == 1 and qc != zn_q[0]:
                                znorm(zn_q.pop(0))
                            prev = (qc, hp, exP)
                        out_heads(*prev)
                        zn_q.append(prev[0])
                        for qc in zn_q:
                            znorm(qc)

                        # out-proj + residual, then LN1
                        res = resp.tile([128, 4, N], bf16, tag="res", name="res")
                        for m in range(4):
                            for (n0, nn) in NT3:
                                ps = pmm.tile([128, 512], f32, tag="mm", name="ps")
                                mm_acc(ps[:, 0:nn],
                                       [(wo_sb[:, j, m * 128:(m + 1) * 128],
                                         oT[:, j, n0:n0 + nn]) for j in range(4)])
                                nc.vector.tensor_add(res[:, m, n0:n0 + nn], ps[:, 0:nn],
                                                     h[:, m, n0:n0 + nn])
                                nc.vector.tensor_scalar(res[:, m, n0:n0 + nn],
                                                        res[:, m, n0:n0 + nn],
                                                        bo_c[:, m:m + 1], None,
                                                        op0=ALU.add)
                        pend_ln = (res, b)
                    h_st[pend_ln[1]] = _layernorm(sqp, mrow, pmm, pot, pend_ln[0],
                                                  g1_c, be1_c, pend_ln[1])

                # ---------- FF + LN2 (freq DFT consts prefetch during FF) ----------
                fcp = tc.alloc_tile_pool(name="fcst", bufs=1)
                ccos_sb = fcp.tile([128, 9, NF], bf16)
                csin_sb = fcp.tile([128, 9, NF], bf16)
                bmt_sb = fcp.tile([128, 5, N], bf16)

                with tc.tile_pool(name="wff", bufs=1) as wp, \
                     tc.tile_pool(name="zp", bufs=2) as zp, \
                     tc.tile_pool(name="sqp2", bufs=1) as sqp, \
                     tc.tile_pool(name="mrow2", bufs=2) as mrow, \
                     tc.tile_pool(name="pmm2", bufs=4, space=bass.MemorySpace.PSUM) as pmm, \
                     tc.tile_pool(name="pot2", bufs=2, space=bass.MemorySpace.PSUM) as pot:

                    w1_sb = wp.tile([128, 4, DFF], bf16)
                    nc.sync.dma_start(out=w1_sb[:], in_=d_w1[l])
                    w2_sb = wp.tile([128, 16, D], bf16)
                    nc.sync.dma_start(out=w2_sb[:], in_=d_w2[l])
                    b1_c = wp.tile([128, 16], f32)
                    nc.sync.dma_start(out=b1_c[:], in_=d_b1[l])
                    b2_c = wp.tile([128, 4], f32)
                    nc.sync.dma_start(out=b2_c[:], in_=d_b2[l])
                    g2_c = wp.tile([128, 4], f32)
                    nc.sync.dma_start(out=g2_c[:], in_=d_g2[l])
                    be2_c = wp.tile([128, 4], f32)
                    nc.sync.dma_start(out=be2_c[:], in_=d_be2[l])
                    # freq consts stream in behind the FF weights
                    nc.sync.dma_start(out=ccos_sb[:], in_=d_ccos)
                    nc.sync.dma_start(out=csin_sb[:], in_=d_csin)
                    nc.sync.dma_start(out=bmt_sb[:], in_=d_bmt)

                    for b in range(BPC):
                        h1 = h_st[b]
                        res = resp.tile([128, 4, N], bf16, tag="res", name="res")
                        for (n0, nn) in NT3:
                            z_sb = zp.tile([128, 16, 512], bf16, tag="z", name="z_sb")
                            for m16 in range(16):
                                ps = pmm.tile([128, 512], f32, tag="mm", name="ps")
                                mm_acc(ps[:, 0:nn],
                                       [(w1_sb[:, j, m16 * 128:(m16 + 1) * 128],
                                         h1[:, j, n0:n0 + nn]) for j in range(4)])
                                nc.scalar.activation(z_sb[:, m16, 0:nn], ps[:, 0:nn],
                                                     AF.Relu, bias=b1_c[:, m16:m16 + 1])
                            for m in range(4):
                                ps = pmm.tile([128, 512], f32, tag="mm", name="ps")
                                mm_acc(ps[:, 0:nn],
                                       [(w2_sb[:, k, m * 128:(m + 1) * 128],
                                         z_sb[:, k, 0:nn]) for k in range(16)])
                                nc.vector.tensor_add(res[:, m, n0:n0 + nn], ps[:, 0:nn],
                                                     h1[:, m, n0:n0 + nn])
                                nc.vector.tensor_scalar(res[:, m, n0:n0 + nn],
                                                        res[:, m, n0:n0 + nn],
                                                        b2_c[:, m:m + 1], None,
                                                        op0=ALU.add)
                        h_st[b] = _layernorm(sqp, mrow, pmm, pot, res, g2_c, be2_c, b)

                # ---------- frequency block ----------
                with tc.tile_pool(name="wfr", bufs=1) as wp, \
                     tc.tile_pool(name="afr", bufs=1) as ap_, \
                     tc.tile_pool(name="afr2", bufs=1) as ap2, \
                     tc.tile_pool(name="frow", bufs=2) as frow, \
                     tc.tile_pool(name="pmm3", bufs=3, space=bass.MemorySpace.PSUM) as pmm, \
                     tc.tile_pool(name="ptr3", bufs=2, space=bass.MemorySpace.PSUM) as ptr, \
                     tc.tile_pool(name="pbc3", bufs=2, space=bass.MemorySpace.PSUM) as pbc:

                    fqw_sb = wp.tile([128, 4, D], bf16)
                    nc.sync.dma_start(out=fqw_sb[:], in_=d_fqw[l])
                    fkw_sb = wp.tile([128, 4, D], bf16)
                    nc.sync.dma_start(out=fkw_sb[:], in_=d_fkw[l])
                    fvw_sb = wp.tile([128, 4, D], bf16)
                    nc.sync.dma_start(out=fvw_sb[:], in_=d_fvw[l])
                    fqb_c = wp.tile([128, 4], f32)
                    nc.sync.dma_start(out=fqb_c[:], in_=d_fqb[l])
                    fkb_c = wp.tile([128, 4], f32)
                    nc.sync.dma_start(out=fkb_c[:], in_=d_fkb[l])
                    fvb_r = wp.tile([1, D], bf16)
                    nc.sync.dma_start(out=fvb_r[:], in_=d_fvb[l])
                    bsum_sb = wp.tile([128, 5, 1], bf16)
                    nc.sync.dma_start(out=bsum_sb[:], in_=d_bsum)

                    for b in range(BPC):
                        h2 = h_st[b]
                        # h in time-on-partition layout via PE transposes
                        htp = ap2.tile([128, 9, D], bf16, tag="htp", name="htp")
                        for tc9, (t0, tn) in enumerate(T9):
                            for j in range(4):
                                pst = ptr.tile([128, 128], bf16, tag="tp", name="pst")
                                nc.tensor.transpose(pst[0:tn, :], h2[:, j, t0:t0 + tn],
                                                    ident[:])
                                nc.scalar.copy(htp[0:tn, tc9, j * 128:(j + 1) * 128],
                                               pst[0:tn, :])
                        # DFT
                        reT = ap2.tile([128, 4, NF], bf16, tag="reT", name="reT")
                        imT = ap2.tile([128, 4, NF], bf16, tag="imT", name="imT")
                        for m in range(4):
                            for (f0, fn) in NF2:
                                ps = pmm.tile([128, 512], f32, tag="mm", name="ps")
                                mm_acc(ps[:, 0:fn],
                                       [(htp[0:tn, i, m * 128:(m + 1) * 128],
                                         ccos_sb[0:tn, i, f0:f0 + fn])
                                        for i, (t0, tn) in enumerate(T9)])
                                nc.scalar.copy(reT[:, m, f0:f0 + fn], ps[:, 0:fn])
                                ps = pmm.tile([128, 512], f32, tag="mm", name="ps")
                                mm_acc(ps[:, 0:fn],
                                       [(htp[0:tn, i, m * 128:(m + 1) * 128],
                                         csin_sb[0:tn, i, f0:f0 + fn])
                                        for i, (t0, tn) in enumerate(T9)])
                                nc.scalar.copy(imT[:, m, f0:f0 + fn], ps[:, 0:fn])
                        # amplitudes -> top-16 mask (sqrt via exp(0.5 ln))
                        absT = ap2.tile([128, 4, NF], bf16, tag="absT", name="absT")
                        tmpT = ap2.tile([128, 4, NF], bf16, tag="tmpT", name="tmpT")
                        lnT = ap2.tile([128, 4, NF], f32, tag="lnT", name="lnT")
                        nc.vector.tensor_mul(absT[:], reT[:], reT[:])
                        nc.vector.tensor_mul(tmpT[:], imT[:], imT[:])
                        nc.vector.tensor_add(absT[:], absT[:], tmpT[:])
                        # sqrt(x) = exp(0.5 ln(x + eps)); the +eps only regularizes
                        # near-zero amps and preserves the top-k ordering exactly
                        nc.scalar.activation(lnT[:], absT[:], AF.Ln, bias=eps_c[:, 0:1])
                        nc.scalar.activation(absT[:], lnT[:], AF.Exp, scale=0.5)
                        amp_row = frow.tile([1, NF], f32, tag="amp", name="amp_row")
                        for (f0, fn) in NF2:
                            ps = pmm.tile([128, 512], f32, tag="mm", name="ps")
                            mm_acc(ps[0:1, 0:fn],
                                   [(ones_col[:], absT[:, j, f0:f0 + fn]) for j in range(4)])
                            nc.scalar.copy(amp_row[0:1, f0:f0 + fn], ps[0:1, 0:fn])
                        work = frow.tile([1, NF], f32, tag="work", name="work")
                        nc.vector.tensor_copy(work[:], amp_row[:])
                        mx8 = frow.tile([1, 8], f32, tag="mx8", name="mx8")
                        for _ in range(2):
                            nc.vector.max(mx8[:], work[:])
                            nc.vector.match_replace(work[:], in_to_replace=mx8[:],
                                                    in_values=work[:], imm_value=0.0)
                        m_row = frow.tile([1, NF], f32, tag="mrow", name="m_row")
                        nc.vector.tensor_sub(m_row[:], amp_row[:], work[:])
                        nc.vector.tensor_scalar(m_row[:], m_row[:], 0.0, None, op0=ALU.is_gt)
                        pen_row = frow.tile([1, NF], bf16, tag="pen", name="pen_row")
                        nc.vector.tensor_scalar(pen_row[:], m_row[:], 1e9, -1e9,
                                                op0=ALU.mult, op1=ALU.add)
                        mb_row = frow.tile([1, NF], bf16, tag="mbrow", name="mb_row")
                        nc.vector.tensor_copy(mb_row[:], m_row[:])
                        # broadcast penalty row; mask column
                        pb_sb = ap2.tile([128, NF], f32, tag="pbsb", name="pb_sb")
                        for (f0, fn) in NF2:
                            pbp = pbc.tile([128, 512], f32, tag="bc", name="pbp")
                            nc.tensor.matmul(pbp[:, 0:fn], ones_row[:],
                                             pen_row[0:1, f0:f0 + fn], start=True, stop=True)
                            nc.vector.tensor_copy(pb_sb[:, f0:f0 + fn], pbp[:, 0:fn])
                        mcol = frow.tile([128, 5], f32, tag="mcol", name="mcol")
                        for fc, (f0, fn) in enumerate(F5):
                            pbp = pbc.tile([128, 512], f32, tag="bc", name="pbp")
                            nc.tensor.matmul(pbp[0:fn, 0:1], mb_row[0:1, f0:f0 + fn],
                                             ones_row[0:1, 0:1], start=True, stop=True)
                            nc.vector.tensor_copy(mcol[0:fn, fc:fc + 1], pbp[0:fn, 0:1])
                        # Q,K (feature-major) and V (freq-major)
                        qT = ap2.tile([128, 4, NF], bf16, tag="qT", name="qT")
                        kTf = ap2.tile([128, 4, NF], bf16, tag="kTf", name="kTf")
                        for m in range(4):
                            for (f0, fn) in NF2:
                                ps = pmm.tile([128, 512], f32, tag="mm", name="ps")
                                mm_acc(ps[:, 0:fn],
                                       [(fqw_sb[:, j, m * 128:(m + 1) * 128],
                                         reT[:, j, f0:f0 + fn]) for j in range(4)])
                                nc.scalar.activation(qT[:, m, f0:f0 + fn], ps[:, 0:fn],
                                                     AF.Identity, bias=fqb_c[:, m:m + 1])
                                ps = pmm.tile([128, 512], f32, tag="mm", name="ps")
                                mm_acc(ps[:, 0:fn],
                                       [(fkw_sb[:, j, m * 128:(m + 1) * 128],
                                         reT[:, j, f0:f0 + fn]) for j in range(4)])
                                nc.scalar.activation(kTf[:, m, f0:f0 + fn], ps[:, 0:fn],
                                                     AF.Identity, bias=fkb_c[:, m:m + 1])
                        v_sb = ap2.tile([128, 5, D], bf16, tag="vfr", name="v_sb")
                        for fc, (f0, fn) in enumerate(F5):
                            ps = pmm.tile([128, 512], f32, tag="mm", name="ps")
                            for j in range(4):
                                nc.tensor.matmul(ps[0:fn, :], reT[:, j, f0:f0 + fn],
                                                 fvw_sb[:, j, :], start=(j == 0), stop=False)
                            nc.tensor.matmul(ps[0:fn, :], ones_row[0:1, 0:fn], fvb_r[:],
                                             start=False, stop=True)
                            nc.scalar.copy(v_sb[0:fn, fc, :], ps[0:fn, :])
                        # masked scores -> softmax (with max subtraction)
                        sc_sb = ap_.tile([128, 5, NF], f32, tag="sc", name="sc_sb")
                        ex_sb = ap2.tile([128, 5, NF], bf16, tag="exf", name="ex_sb")
                        zcol = frow.tile([128, 5], f32, tag="zcol", name="zcol")
                        ncol = frow.tile([128, 5], f32, tag="ncol", name="ncol")
                        for qc, (q0, qn) in enumerate(F5):
                            for (f0, fn) in NF2:
                                ps = pmm.tile([128, 512], f32, tag="mm", name="ps")
                                mm_acc(ps[0:qn, 0:fn],
                                       [(qT[:, j, q0:q0 + qn], kTf[:, j, f0:f0 + fn])
                                        for j in range(4)])
                                nc.vector.tensor_add(sc_sb[0:qn, qc, f0:f0 + fn],
                                                     ps[0:qn, 0:fn], pb_sb[0:qn, f0:f0 + fn])
                            nc.vector.reduce_max(ncol[0:qn, qc:qc + 1], sc_sb[0:qn, qc, :],
                                                 axis=AX.X, negate=True)
                            nc.scalar.activation(ex_sb[0:qn, qc, :], sc_sb[0:qn, qc, :],
                                                 AF.Exp, bias=ncol[0:qn, qc:qc + 1],
                                                 accum_out=zcol[0:qn, qc:qc + 1])
                        rinv = frow.tile([128, 5], f32, tag="rinvf", name="rinv")
                        nc.vector.reciprocal_approx_fast(rinv[:], zcol[:])
                        wcol = frow.tile([128, 5], f32, tag="wcol", name="wcol")
                        nc.vector.tensor_mul(wcol[:], rinv[:], mcol[:])
                        # transpose exp -> (k_f, q_f)
                        exT = ap2.tile([128, 5, NF], bf16, tag="exT", name="exT")
                        for qc, (q0, qn) in enumerate(F5):
                            for fc, (f0, fn) in enumerate(F5):
                                pst = ptr.tile([128, 128], bf16, tag="tp", name="pst")
                                nc.tensor.transpose(pst[0:fn, 0:qn],
                                                    ex_sb[0:qn, qc, f0:f0 + fn],
                                                    ident[0:qn, 0:qn])
                                nc.scalar.copy(exT[0:fn, fc, q0:q0 + qn], pst[0:fn, 0:qn])
                        # ctx = attn @ V, masked+normalized
                        ctxm = ap2.tile([128, 5, D], bf16, tag="ctxm", name="ctxm")
                        for qc, (q0, qn) in enumerate(F5):
                            ps = pmm.tile([128, 512], f32, tag="mm", name="ps")
                            mm_acc(ps[0:qn, :],
                                   [(exT[0:fn, fc, q0:q0 + qn], v_sb[0:fn, fc, :])
                                    for fc, (f0, fn) in enumerate(F5)])
                            nc.vector.tensor_scalar(ctxm[0:qn, qc, :], ps[0:qn, :],
                                                    wcol[0:qn, qc:qc + 1], None,
                                                    op0=ALU.mult)
                        if l < L - 1:
                            # irfft: h_next = B @ ctxm  (feature-major out)
                            hn = statep.tile([128, 4, N], bf16, tag=f"h{b}", name="hn")
                            for m in range(4):
                                for (n0, nn) in NT3:
                                    ps = pmm.tile([128, 512], f32, tag="mm", name="ps")
                                    mm_acc(ps[:, 0:nn],
                                           [(ctxm[0:fn, fc, m * 128:(m + 1) * 128],
                                             bmt_sb[0:fn, fc, n0:n0 + nn])
                                            for fc, (f0, fn) in enumerate(F5)])
                                    nc.scalar.copy(hn[:, m, n0:n0 + nn], ps[:, 0:nn])
                            h_st[b] = hn
                        else:
                            # final layer: only mean_t(irfft) is needed ->
                            # pooled = (colsum B) @ ctxm, a K=525 N=1 matmul
                            ps = pbc.tile([128, 512], f32, tag="bc", name="ps")
                            for m in range(4):
                                for fc, (f0, fn) in enumerate(F5):
                                    nc.tensor.matmul(
                                        ps[:, m:m + 1],
                                        ctxm[0:fn, fc, m * 128:(m + 1) * 128],
                                        bsum_sb[0:fn, fc, :],
                                        start=(fc == 0), stop=(fc == 4))
                            nc.vector.tensor_copy(poolh[:, b, :], ps[:, 0:4])
                fcp.release()

            # ================= head =================
            with tc.tile_pool(name="hd", bufs=1) as hp, \
                 tc.tile_pool(name="hd2", bufs=2) as hp2, \
                 tc.tile_pool(name="phd", bufs=2, space=bass.MemorySpace.PSUM) as php:
                hw_sb = hp.tile([128, 4, OUT_LEN * N_TGT], f32)
                nc.sync.dma_start(out=hw_sb[:], in_=d_hw)
                hb_sb = hp.tile([1, OUT_LEN * N_TGT], f32)
                nc.sync.dma_start(out=hb_sb[:], in_=d_hb)
                for b in range(BPC):
                    pool_c = hp2.tile([128, 4], f32, tag="pool", name="pool_c")
                    nc.vector.tensor_add(pool_c[:], poolh[:, b, :], pool2[:, b, :])
                    ps = php.tile([1, OUT_LEN * N_TGT], f32, tag="y", name="ps")
                    for j in range(4):
                        nc.tensor.matmul(ps[:], pool_c[:, j:j + 1], hw_sb[:, j, :],
                                         start=(j == 0), stop=False)
                    nc.tensor.matmul(ps[:], ones11f[0:1, 0:1], hb_sb[:],
                                     start=False, stop=True)
                    y_sb = hp2.tile([1, OUT_LEN * N_TGT], f32, tag="ysb", name="y_sb")
                    nc.scalar.copy(y_sb[:], ps[:])
                    nc.sync.dma_start(out=d_y[b:b + 1, :], in_=y_sb[:])

    nc.compile()
    return nc


_NC_CACHE = {}


def _get_nc():
    if "nc" not in _NC_CACHE:
        _NC_CACHE["nc"] = _build()
    return _NC_CACHE["nc"]


def _tile_w(w):
    # (L, rows, cols) -> (L, 128, rows//128, cols)
    Lc, rows, cols = w.shape
    return np.ascontiguousarray(
        w.reshape(Lc, rows // 128, 128, cols).transpose(0, 2, 1, 3))


def _col_b(v):
    # (L, m*128) -> (L, 128, m)
    Lc, n = v.shape
    return np.ascontiguousarray(v.reshape(Lc, n // 128, 128).transpose(0, 2, 1))


def _prepare_in_maps(inputs):
    x = np.asarray(inputs["x"], np.float32)
    sq8 = 1.0 / 8.0
    sqD = 1.0 / np.sqrt(np.float32(D))
    qkv_w = np.asarray(inputs["qkv_w"], np.float32).copy()
    qkv_b = np.asarray(inputs["qkv_b"], np.float32).copy()
    qkv_w[:, :, :D] *= sq8
    qkv_b[:, :D] *= sq8
    fq_w = np.asarray(inputs["fq_w"], np.float32) * sqD
    fq_b = np.asarray(inputs["fq_b"], np.float32) * sqD
    ccos, csin, bmt, at, bsum, peT = _HOST_CONSTS
    b_in = np.asarray(inputs["b_in"], np.float32)
    peb = (peT + b_in[:, None]).astype(BF)                       # (512, 1048) bf16
    peb = np.ascontiguousarray(
        peb.reshape(4, 128, N).transpose(1, 0, 2))               # (128, 4, 1048)
    head_w = np.asarray(inputs["head_w"], np.float32)
    hb = (b_in @ head_w + np.asarray(inputs["head_b"], np.float32))[None, :]
    hw = head_w / np.float32(N)                                  # (512, 192)
    hw = np.ascontiguousarray(hw.reshape(4, 128, -1).transpose(1, 0, 2))

    common = {
        "win": np.asarray(inputs["Win"], np.float32).astype(BF),
        "wqk": _tile_w(qkv_w[:, :, :2 * D].astype(BF)),
        "bqk": _col_b(np.ascontiguousarray(qkv_b[:, :2 * D])),
        "wv": _tile_w(np.ascontiguousarray(qkv_w[:, :, 2 * D:]).astype(BF)),
        "bv": np.ascontiguousarray(qkv_b[:, None, 2 * D:]).astype(BF),
        "wo": _tile_w(np.asarray(inputs["out_w"], np.float32).astype(BF)),
        "bo": _col_b(np.asarray(inputs["out_b"], np.float32)),
        "w1": _tile_w(np.asarray(inputs["ff_w1"], np.float32).astype(BF)),
        "b1": _col_b(np.asarray(inputs["ff_b1"], np.float32)),
        "w2": _tile_w(np.asarray(inputs["ff_w2"], np.float32).astype(BF)),
        "b2": _col_b(np.asarray(inputs["ff_b2"], np.float32)),
        "g1": _col_b(np.asarray(inputs["ln1_g"], np.float32)),
        "be1": _col_b(np.asarray(inputs["ln1_b"], np.float32)),
        "g2": _col_b(np.asarray(inputs["ln2_g"], np.float32)),
        "be2": _col_b(np.asarray(inputs["ln2_b"], np.float32)),
        "fqw": _tile_w(fq_w.astype(BF)),
        "fqb": _col_b(fq_b),
        "fkw": _tile_w(np.asarray(inputs["fk_w"], np.float32).astype(BF)),
        "fkb": _col_b(np.asarray(inputs["fk_b"], np.float32)),
        "fvw": _tile_w(np.asarray(inputs["fv_w"], np.float32).astype(BF)),
        "fvb": np.asarray(inputs["fv_b"], np.float32)[:, None, :].astype(BF),
        "ccos": ccos, "csin": csin, "bmt": bmt, "at": at, "bsum": bsum,
        "peb": peb, "hw": hw, "hb": hb,
    }
    in_maps = []
    for c in range(NCORES):
        m = dict(common)
        xc = x[c * BPC:(c + 1) * BPC].astype(BF)                 # (BPC, 1024, 64)
        m["x2"] = np.ascontiguousarray(
            xc.reshape(BPC, 8, 128, F_IN).transpose(0, 2, 1, 3))
        in_maps.append(m)
    return in_maps


def kernel(**inputs):
    in_maps = _prepare_in_maps(inputs)
    from concourse.bass_utils import run_bass_kernel_spmd
    nc = _get_nc()
    res = run_bass_kernel_spmd(nc, in_maps, core_ids=list(range(NCORES)))
    ys = np.concatenate([res.results[c]["y"] for c in range(NCORES)], axis=0)
    return ys.reshape(B, OUT_LEN, N_TGT).astype(np.float32)


_HOST_CONSTS = _host_constants()
